# revision 2
# baseline (speedup 1.0000x reference)
"""GraphTripleConv on 8 Trainium2 NeuronCores (Bass/Tile).

Sharding: core k owns objects [6250k, 6250(k+1)). Each core runs net1 over
every triple whose subject OR object it owns (union stream, ~47k triples),
pools new_s/new_o contributions for its own objects via destination-sorted
one-hot matmuls, and runs net2 over its object slice. No collectives.
"""
import sys
import numpy as np

sys.path.insert(0, "/opt/trn_rl_repo")

N_OBJ = 50000
T = 200000
DIN = 128
H = 512
DOUT = 128
NCORES = 8
NPC = N_OBJ // NCORES          # 6250 objects per core
NBLK = 49                      # ceil(6250/128) object blocks
NPC_PAD = NBLK * 128           # 6272
R_PAD = 47104                  # padded union-stream length (max observed 46948)
NTILES = R_PAD // 128          # 368
S_CH = 5                       # s-contribution chunks per block (max 576 <= 640)
O_CH = 5                       # o-contribution chunks per block (max 579 <= 640)
SO_ROWS = 51201                # staging rows: n_s+n_o (<=51200) + 1 trash row
TRASH = SO_ROWS - 1

_PROGRAM = None


def _build_program():
    import concourse.bass as bass
    import concourse.mybir as mybir
    import concourse.tile as tile
    from concourse import bacc
    from concourse.masks import make_identity

    P = 128
    f32 = mybir.dt.float32
    i32 = mybir.dt.int32
    AF = mybir.ActivationFunctionType

    nc = bacc.Bacc("TRN2", target_bir_lowering=False, debug=False,
                   num_devices=NCORES)

    def din(name, shape, dt=f32):
        return nc.dram_tensor(name, shape, dt, kind="ExternalInput").ap()

    t_obj = din("obj_vecs", [N_OBJ, DIN])
    t_predT = din("predT", [P, R_PAD])
    t_sg = din("sg_idx", [P, NTILES], i32)
    t_og = din("og_idx", [P, NTILES], i32)
    t_sslot = din("s_slot", [P, NTILES], i32)
    t_oslot = din("o_slot", [P, NTILES], i32)
    t_sgath = din("s_gath", [P, NBLK * S_CH], i32)
    t_soh = din("s_oh", [P, NBLK * S_CH])
    t_ogath = din("o_gath", [P, NBLK * O_CH], i32)
    t_ooh = din("o_oh", [P, NBLK * O_CH])
    t_inv = din("invcnt", [P, NBLK])
    t_W1a = din("W1a", [3 * P, H])
    t_b1a = din("b1a", [1, H])
    t_W1b = din("W1b", [H, 2 * H + DOUT])
    t_b1b = din("b1b", [1, 2 * H + DOUT])
    t_W2a = din("W2a", [H, H])
    t_b2a = din("b2a", [1, H])
    t_W2b = din("W2b", [H, DOUT])
    t_b2b = din("b2b", [1, DOUT])

    o_newp = nc.dram_tensor("new_p_shard", [R_PAD, DOUT], f32,
                            kind="ExternalOutput").ap()
    o_newo = nc.dram_tensor("new_obj_shard", [NPC_PAD, DOUT], f32,
                            kind="ExternalOutput").ap()
    stage = nc.dram_tensor("stage_so", [SO_ROWS, H], f32).ap()

    with tile.TileContext(nc) as tc:
        with tc.tile_pool(name="const", bufs=1) as cp:
            ident = cp.tile([P, P], f32, tag="ident")
            make_identity(nc, ident[:])
            iota_i = cp.tile([P, P], i32, tag="iota_i")
            nc.gpsimd.iota(iota_i[:], pattern=[[1, P]], base=0,
                           channel_multiplier=0)
            iota_f = cp.tile([P, P], f32, tag="iota_f")
            nc.vector.tensor_copy(iota_f[:], iota_i[:])
            ones = cp.tile([1, P], f32, tag="ones")
            nc.gpsimd.memset(ones[:], 1.0)

            w1a = cp.tile([P, 3, H], f32, tag="w1a")
            nc.sync.dma_start(out=w1a[:], in_=t_W1a.rearrange("(c p) n -> p c n", p=P))
            w1b = cp.tile([P, 4, 2 * H + DOUT], f32, tag="w1b")
            nc.sync.dma_start(out=w1b[:], in_=t_W1b.rearrange("(c p) n -> p c n", p=P))
            w2a = cp.tile([P, 4, H], f32, tag="w2a")
            nc.sync.dma_start(out=w2a[:], in_=t_W2a.rearrange("(c p) n -> p c n", p=P))
            w2b = cp.tile([P, 4, DOUT], f32, tag="w2b")
            nc.sync.dma_start(out=w2b[:], in_=t_W2b.rearrange("(c p) n -> p c n", p=P))
            b1a = cp.tile([1, H], f32, tag="b1a")
            nc.sync.dma_start(out=b1a[:], in_=t_b1a[:])
            b1b = cp.tile([1, 2 * H + DOUT], f32, tag="b1b")
            nc.sync.dma_start(out=b1b[:], in_=t_b1b[:])
            b2a = cp.tile([1, H], f32, tag="b2a")
            nc.sync.dma_start(out=b2a[:], in_=t_b2a[:])
            b2b = cp.tile([1, DOUT], f32, tag="b2b")
            nc.sync.dma_start(out=b2b[:], in_=t_b2b[:])

            sg = cp.tile([P, NTILES], i32, tag="sg")
            nc.sync.dma_start(out=sg[:], in_=t_sg[:])
            og = cp.tile([P, NTILES], i32, tag="og")
            nc.sync.dma_start(out=og[:], in_=t_og[:])
            sslot = cp.tile([P, NTILES], i32, tag="sslot")
            nc.sync.dma_start(out=sslot[:], in_=t_sslot[:])
            oslot = cp.tile([P, NTILES], i32, tag="oslot")
            nc.sync.dma_start(out=oslot[:], in_=t_oslot[:])
            sgath = cp.tile([P, NBLK * S_CH], i32, tag="sgath")
            nc.sync.dma_start(out=sgath[:], in_=t_sgath[:])
            soh = cp.tile([P, NBLK * S_CH], f32, tag="soh")
            nc.sync.dma_start(out=soh[:], in_=t_soh[:])
            ogath = cp.tile([P, NBLK * O_CH], i32, tag="ogath")
            nc.sync.dma_start(out=ogath[:], in_=t_ogath[:])
            ooh = cp.tile([P, NBLK * O_CH], f32, tag="ooh")
            nc.sync.dma_start(out=ooh[:], in_=t_ooh[:])
            inv = cp.tile([P, NBLK], f32, tag="inv")
            nc.sync.dma_start(out=inv[:], in_=t_inv[:])

            # ---------------- Phase 1: net1 over the triple stream ----------
            with (
                tc.tile_pool(name="p1sb", bufs=1) as sb,
                tc.tile_pool(name="p1ps", bufs=1, space="PSUM") as pp,
            ):
                for it in range(NTILES):
                    rs = slice(it * P, (it + 1) * P)
                    s_rows = sb.tile([P, DIN], f32, tag="srows", bufs=3)
                    nc.gpsimd.indirect_dma_start(
                        out=s_rows[:], out_offset=None, in_=t_obj[:],
                        in_offset=bass.IndirectOffsetOnAxis(
                            ap=sg[:, it:it + 1], axis=0))
                    o_rows = sb.tile([P, DIN], f32, tag="orows", bufs=3)
                    nc.gpsimd.indirect_dma_start(
                        out=o_rows[:], out_offset=None, in_=t_obj[:],
                        in_offset=bass.IndirectOffsetOnAxis(
                            ap=og[:, it:it + 1], axis=0))
                    pT = sb.tile([P, P], f32, tag="pchunk", bufs=3)
                    nc.sync.dma_start(out=pT[:], in_=t_predT[:, rs])

                    # cur_t^T chunks: subject^T, pred^T, object^T
                    xT = []
                    for src in (s_rows, o_rows):
                        tp = pp.tile([P, P], f32, tag="tp", bufs=2, space="PSUM")
                        nc.tensor.transpose(out=tp[:], in_=src[:], identity=ident[:])
                        xt = sb.tile([P, P], f32, tag="xT", bufs=4)
                        nc.vector.tensor_copy(xt[:], tp[:])
                        xT.append(xt)
                    chunks = [xT[0], pT, xT[1]]

                    h_ps = pp.tile([P, H], f32, tag="hps", bufs=2, space="PSUM")
                    nc.tensor.matmul(out=h_ps[:], lhsT=ones[:1, :], rhs=b1a[:1, :],
                                     start=True, stop=False)
                    for c in range(3):
                        nc.tensor.matmul(out=h_ps[:], lhsT=chunks[c][:],
                                         rhs=w1a[:, c, :], start=False,
                                         stop=(c == 2))
                    h_sb = sb.tile([P, H], f32, tag="hsb", bufs=2)
                    nc.scalar.activation(h_sb[:], h_ps[:], AF.Relu)

                    hT = []
                    for c in range(4):
                        tp = pp.tile([P, P], f32, tag="tp", bufs=2, space="PSUM")
                        nc.tensor.transpose(out=tp[:], in_=h_sb[:, c * P:(c + 1) * P],
                                            identity=ident[:])
                        ht = sb.tile([P, P], f32, tag="hT", bufs=6)
                        nc.vector.tensor_copy(ht[:], tp[:])
                        hT.append(ht)

                    ns_ps = pp.tile([P, H], f32, tag="nsps", bufs=1, space="PSUM")
                    np_ps = pp.tile([P, DOUT], f32, tag="npps", bufs=1, space="PSUM")
                    no_ps = pp.tile([P, H], f32, tag="nops", bufs=1, space="PSUM")
                    nc.tensor.matmul(out=ns_ps[:], lhsT=ones[:1, :],
                                     rhs=b1b[:1, 0:H], start=True, stop=False)
                    nc.tensor.matmul(out=np_ps[:], lhsT=ones[:1, :],
                                     rhs=b1b[:1, H:H + DOUT], start=True, stop=False)
                    nc.tensor.matmul(out=no_ps[:], lhsT=ones[:1, :],
                                     rhs=b1b[:1, H + DOUT:], start=True, stop=False)
                    for c in range(4):
                        last = c == 3
                        nc.tensor.matmul(out=ns_ps[:], lhsT=hT[c][:],
                                         rhs=w1b[:, c, 0:H], start=False, stop=last)
                        nc.tensor.matmul(out=np_ps[:], lhsT=hT[c][:],
                                         rhs=w1b[:, c, H:H + DOUT], start=False,
                                         stop=last)
                        nc.tensor.matmul(out=no_ps[:], lhsT=hT[c][:],
                                         rhs=w1b[:, c, H + DOUT:], start=False,
                                         stop=last)

                    ns_sb = sb.tile([P, H], f32, tag="nssb", bufs=2)
                    nc.scalar.activation(ns_sb[:], ns_ps[:], AF.Relu)
                    np_sb = sb.tile([P, DOUT], f32, tag="npsb", bufs=2)
                    nc.scalar.activation(np_sb[:], np_ps[:], AF.Relu)
                    no_sb = sb.tile([P, H], f32, tag="nosb", bufs=2)
                    nc.scalar.activation(no_sb[:], no_ps[:], AF.Relu)

                    nc.sync.dma_start(out=o_newp[rs, :], in_=np_sb[:])
                    nc.gpsimd.indirect_dma_start(
                        out=stage[:], out_offset=bass.IndirectOffsetOnAxis(
                            ap=sslot[:, it:it + 1], axis=0),
                        in_=ns_sb[:], in_offset=None)
                    nc.gpsimd.indirect_dma_start(
                        out=stage[:], out_offset=bass.IndirectOffsetOnAxis(
                            ap=oslot[:, it:it + 1], axis=0),
                        in_=no_sb[:], in_offset=None)

            # ---------------- Phase 2: pool + net2 per object block ---------
            with (
                tc.tile_pool(name="p2sb", bufs=1) as sb,
                tc.tile_pool(name="p2ps", bufs=1, space="PSUM") as pp,
            ):
                for b in range(NBLK):
                    pool_ps = pp.tile([P, H], f32, tag="plps", bufs=2, space="PSUM")
                    first = True
                    for side, nch, gath, oh in (
                        ("s", S_CH, sgath, soh),
                        ("o", O_CH, ogath, ooh),
                    ):
                        for c in range(nch):
                            col = b * nch + c
                            rows_t = sb.tile([P, H], f32, tag="grows", bufs=4)
                            nc.gpsimd.indirect_dma_start(
                                out=rows_t[:], out_offset=None, in_=stage[:],
                                in_offset=bass.IndirectOffsetOnAxis(
                                    ap=gath[:, col:col + 1], axis=0))
                            onehot = sb.tile([P, P], f32, tag="onehot", bufs=4)
                            nc.vector.tensor_tensor(
                                out=onehot[:],
                                in0=oh[:, col:col + 1].to_broadcast([P, P]),
                                in1=iota_f[:], op=mybir.AluOpType.is_equal)
                            nc.tensor.matmul(out=pool_ps[:], lhsT=onehot[:],
                                             rhs=rows_t[:], start=first,
                                             stop=(side == "o" and c == nch - 1))
                            first = False
                    pooled = sb.tile([P, H], f32, tag="pooled", bufs=2)
                    nc.scalar.activation(pooled[:], pool_ps[:], AF.Copy,
                                         scale=inv[:, b:b + 1])

                    pTc = []
                    for c in range(4):
                        tp = pp.tile([P, P], f32, tag="tp2", bufs=2, space="PSUM")
                        nc.tensor.transpose(out=tp[:], in_=pooled[:, c * P:(c + 1) * P],
                                            identity=ident[:])
                        pt = sb.tile([P, P], f32, tag="pT", bufs=5)
                        nc.vector.tensor_copy(pt[:], tp[:])
                        pTc.append(pt)
                    h2_ps = pp.tile([P, H], f32, tag="h2ps", bufs=1, space="PSUM")
                    nc.tensor.matmul(out=h2_ps[:], lhsT=ones[:1, :], rhs=b2a[:1, :],
                                     start=True, stop=False)
                    for c in range(4):
                        nc.tensor.matmul(out=h2_ps[:], lhsT=pTc[c][:],
                                         rhs=w2a[:, c, :], start=False,
                                         stop=(c == 3))
                    h2_sb = sb.tile([P, H], f32, tag="h2sb", bufs=2)
                    nc.scalar.activation(h2_sb[:], h2_ps[:], AF.Relu)

                    h2T = []
                    for c in range(4):
                        tp = pp.tile([P, P], f32, tag="tp2", bufs=2, space="PSUM")
                        nc.tensor.transpose(out=tp[:], in_=h2_sb[:, c * P:(c + 1) * P],
                                            identity=ident[:])
                        ht = sb.tile([P, P], f32, tag="h2T", bufs=5)
                        nc.vector.tensor_copy(ht[:], tp[:])
                        h2T.append(ht)
                    o_ps = pp.tile([P, DOUT], f32, tag="o2ps", bufs=1, space="PSUM")
                    nc.tensor.matmul(out=o_ps[:], lhsT=ones[:1, :], rhs=b2b[:1, :],
                                     start=True, stop=False)
                    for c in range(4):
                        nc.tensor.matmul(out=o_ps[:], lhsT=h2T[c][:],
                                         rhs=w2b[:, c, :], start=False,
                                         stop=(c == 3))
                    out_sb = sb.tile([P, DOUT], f32, tag="outsb", bufs=2)
                    nc.scalar.activation(out_sb[:], o_ps[:], AF.Relu)
                    nc.sync.dma_start(out=o_newo[b * P:(b + 1) * P, :], in_=out_sb[:])

    nc.compile()
    return nc


def _prep_core(k, s, o, pred_vecs):
    """Host-side plan for core k. Returns the per-core input map (sans weights)."""
    i32 = np.int32
    f32 = np.float32
    mask_s = (s // NPC) == k
    mask_o = (o // NPC) == k
    stream = np.flatnonzero(mask_s | mask_o)
    R = len(stream)
    assert R <= R_PAD, (k, R)

    sg = np.zeros(R_PAD, i32)
    og = np.zeros(R_PAD, i32)
    sg[:R] = s[stream]
    og[:R] = o[stream]

    predT = np.zeros((128, R_PAD), f32)
    predT[:, :R] = pred_vecs[stream].T

    # s-side contributions, sorted by destination object
    s_members = np.flatnonzero(mask_s[stream])
    s_loc = s[stream[s_members]] - k * NPC
    order = np.argsort(s_loc, kind="stable")
    s_rows_sorted = s_members[order]
    s_loc_sorted = s_loc[order]
    n_s = len(s_members)
    s_slot = np.full(R_PAD, TRASH, i32)
    s_slot[s_rows_sorted] = np.arange(n_s, dtype=i32)

    o_members = np.flatnonzero(mask_o[stream])
    o_loc = o[stream[o_members]] - k * NPC
    order_o = np.argsort(o_loc, kind="stable")
    o_rows_sorted = o_members[order_o]
    o_loc_sorted = o_loc[order_o]
    n_o = len(o_members)
    assert n_s + n_o <= SO_ROWS - 1, (k, n_s, n_o)
    o_slot = np.full(R_PAD, TRASH, i32)
    o_slot[o_rows_sorted] = n_s + np.arange(n_o, dtype=i32)

    def pool_plan(loc_sorted, nch, base):
        gath = np.zeros((NBLK, nch * 128), i32)
        ohv = np.full((NBLK, nch * 128), 999.0, f32)
        blk = loc_sorted // 128
        starts = np.searchsorted(blk, np.arange(NBLK))
        ends = np.searchsorted(blk, np.arange(NBLK) + 1)
        for b in range(NBLK):
            cnt = ends[b] - starts[b]
            assert cnt <= nch * 128, (k, b, cnt)
            gath[b, :cnt] = base + np.arange(starts[b], ends[b], dtype=i32)
            ohv[b, :cnt] = (loc_sorted[starts[b]:ends[b]] - b * 128).astype(f32)
        # device layout: [128 partitions, NBLK*nch cols]
        return (np.ascontiguousarray(gath.reshape(NBLK * nch, 128).T),
                np.ascontiguousarray(ohv.reshape(NBLK * nch, 128).T))

    s_gath, s_oh = pool_plan(s_loc_sorted, S_CH, 0)
    o_gath, o_oh = pool_plan(o_loc_sorted, O_CH, n_s)

    col = lambda a: np.ascontiguousarray(a.reshape(NTILES, 128).T)
    return {
        "predT": predT,
        "sg_idx": col(sg), "og_idx": col(og),
        "s_slot": col(s_slot), "o_slot": col(o_slot),
        "s_gath": s_gath, "s_oh": s_oh, "o_gath": o_gath, "o_oh": o_oh,
    }, stream, s_members


def kernel(obj_vecs, pred_vecs, edges, W1a, b1a, W1b, b1b, W2a, b2a, W2b, b2b):
    global _PROGRAM
    from concourse.bass_utils import run_bass_kernel_spmd

    obj_vecs = np.ascontiguousarray(obj_vecs, np.float32)
    pred_vecs = np.ascontiguousarray(pred_vecs, np.float32)
    edges = np.asarray(edges)
    s = edges[:, 0].astype(np.int32)
    o = edges[:, 1].astype(np.int32)

    cnt = np.bincount(s, minlength=N_OBJ) + np.bincount(o, minlength=N_OBJ)
    invc = (1.0 / np.maximum(cnt, 1)).astype(np.float32)
    inv_pad = np.ones(NCORES * NPC_PAD, np.float32)

    common = {
        "obj_vecs": obj_vecs,
        "W1a": np.ascontiguousarray(W1a, np.float32),
        "b1a": np.ascontiguousarray(np.asarray(b1a, np.float32)[None, :]),
        "W1b": np.ascontiguousarray(W1b, np.float32),
        "b1b": np.ascontiguousarray(np.asarray(b1b, np.float32)[None, :]),
        "W2a": np.ascontiguousarray(W2a, np.float32),
        "b2a": np.ascontiguousarray(np.asarray(b2a, np.float32)[None, :]),
        "W2b": np.ascontiguousarray(W2b, np.float32),
        "b2b": np.ascontiguousarray(np.asarray(b2b, np.float32)[None, :]),
    }

    in_maps = []
    streams = []
    s_members_l = []
    for k in range(NCORES):
        m, stream, s_members = _prep_core(k, s, o, pred_vecs)
        iv = np.ones(NPC_PAD, np.float32)
        iv[:NPC] = invc[k * NPC:(k + 1) * NPC]
        m["invcnt"] = np.ascontiguousarray(iv.reshape(NBLK, 128).T)
        m.update(common)
        in_maps.append(m)
        streams.append(stream)
        s_members_l.append(s_members)

    if _PROGRAM is None:
        _PROGRAM = _build_program()

    res = run_bass_kernel_spmd(_PROGRAM, in_maps, list(range(NCORES)))
    globals()["_LAST_RES"] = res

    new_p = np.empty((T, DOUT), np.float32)
    new_obj = np.empty((N_OBJ, DOUT), np.float32)
    for k in range(NCORES):
        r = res.results[k]
        sm = s_members_l[k]
        new_p[streams[k][sm]] = r["new_p_shard"][sm]
        new_obj[k * NPC:(k + 1) * NPC] = r["new_obj_shard"][:NPC]
    return new_obj, new_p
